# revision 10
# baseline (speedup 1.0000x reference)
"""AFD channel attention on 8 TRN2 NeuronCores.

Math (per row r of x_flat [B*C, L], L = 64*64 = 4096, N = 64 basis fns):
    proj = x_flat @ B.T            [BC, N]
    w    = softmax(|proj|, -1)     [BC, N]
    out  = x_flat + w @ B          [BC, L]

Strategy: data-parallel over the 16384 (b, c) rows, 2048 rows per core.
Everything on-device runs in the TRANSPOSED domain (outT = xT + attnT) so
that the contraction dim of both matmuls lies on SBUF partitions with no
on-chip transpose of the big tensor: the host ships xT [L, 2048] per core
(bf16), the device returns outT [L, 2048] (bf16), host transposes back.
"""

import sys

for p in ("/opt/trn_rl_repo", "/root/.axon_site/_ro/trn_rl_repo"):
    if p not in sys.path:
        sys.path.append(p)

import numpy as np
import ml_dtypes

import concourse.bass as bass
import concourse.mybir as mybir
import concourse.tile as tile
from concourse.bass_utils import run_bass_kernel_spmd

BF16 = mybir.dt.bfloat16
F32 = mybir.dt.float32
NP_BF16 = ml_dtypes.bfloat16

N_BASIS = 64
R = 0.9
L = 4096            # 64 * 64
BC_TOTAL = 16384    # 32 * 512
N_CORES = 8
BC = BC_TOTAL // N_CORES   # 2048 rows per core
KC = L // 128       # 32 l-chunks of 128
NG = BC // 512      # 4 bc-groups of 512
NT = BC // 128      # 16 bc-tiles of 128


def _blaschke_basis_f64(length):
    thetas = np.linspace(0.0, 2.0 * np.pi, N_BASIS, endpoint=False)
    t = np.linspace(0.0, 2.0 * np.pi, length)
    cosp = np.cos(t[None, :] - thetas[:, None])
    scale = np.sqrt(1.0 - R * R)
    return scale * (1.0 - R * cosp) / (1.0 - 2.0 * R * cosp + R * R)  # [N, L]


NS = 2              # BC slices per core (pipelining granularity)
SBC = BC // NS      # 1024 columns per slice
SNG = SBC // 512    # 2 groups of 512 per slice
SNT = SBC // 128    # 8 softmax tiles per slice


def _build():
    nc = bass.Bass()
    xt_ext = nc.declare_dram_parameter("xt", [L, BC], BF16, isOutput=False)
    bt_ext = nc.declare_dram_parameter("bt", [L, N_BASIS], BF16, isOutput=False)
    bn_ext = nc.declare_dram_parameter("bn", [N_BASIS, L], BF16, isOutput=False)
    id_ext = nc.declare_dram_parameter("ident", [128, 128], F32, isOutput=False)
    out_ext = nc.declare_dram_parameter("out", [L, BC], BF16, isOutput=True)

    add = mybir.AluOpType.add
    X = mybir.AxisListType.X
    Act = mybir.ActivationFunctionType

    with tile.TileContext(nc) as tc:
        with (
            tc.tile_pool(name="xt", bufs=NS * KC) as xt_pool,
            tc.tile_pool(name="const", bufs=1) as cpool,
            tc.tile_pool(name="smproj", bufs=2) as smproj_pool,
            tc.tile_pool(name="sm", bufs=4) as sm_pool,
            tc.tile_pool(name="outp", bufs=4) as out_pool,
            tc.tile_pool(name="ps_proj", bufs=NS * SNG, space="PSUM") as ps_proj,
            tc.tile_pool(name="ps_t", bufs=2, space="PSUM") as ps_t,
            tc.tile_pool(name="ps_attn", bufs=2, space="PSUM") as ps_attn,
        ):
            # -- constants --
            bt_sb = cpool.tile([128, KC * N_BASIS], BF16)   # chunk k at [:, 64k:64k+64]
            bn_sb = cpool.tile([N_BASIS, L], BF16)
            id_sb = cpool.tile([128, 128], F32)
            idbf_sb = cpool.tile([128, 128], BF16)
            nc.sync.dma_start(bn_sb[:], bn_ext[:])
            nc.sync.dma_start(id_sb[:], id_ext[:])
            nc.vector.tensor_copy(idbf_sb[:], id_sb[:])
            for k in range(KC):
                nc.sync.dma_start(
                    bt_sb[:, k * N_BASIS:(k + 1) * N_BASIS],
                    bt_ext[k * 128:(k + 1) * 128, :],
                )

            # -- all input DMAs issued up-front so the SP HWDGE stream never
            #    stalls behind output DMAs of an earlier slice --
            xt_tiles = {}
            for s in range(NS):
                for k in range(KC):
                    xt_t = xt_pool.tile([128, SBC], BF16, tag="xt", name=f"xt{s}_{k}")
                    nc.sync.dma_start(
                        xt_t[:],
                        xt_ext[k * 128:(k + 1) * 128, s * SBC:(s + 1) * SBC],
                    )
                    xt_tiles[(s, k)] = xt_t

            unit = 0  # epilogue path round-robin across DVE / PE+ACT
            for s in range(NS):
                c0 = s * SBC

                # -- phase A: accumulate projT = B @ x.T for this slice --
                proj_ps = [
                    ps_proj.tile([N_BASIS, 512], F32, tag="proj", name=f"proj{s}_{g}")
                    for g in range(SNG)
                ]
                for k in range(KC):
                    for g in range(SNG):
                        nc.tensor.matmul(
                            proj_ps[g][:],
                            bt_sb[:, k * N_BASIS:(k + 1) * N_BASIS],
                            xt_tiles[(s, k)][:, g * 512:(g + 1) * 512],
                            start=(k == 0),
                            stop=(k == KC - 1),
                        )

                # -- phase B: softmax over the 64 coefficients --
                projT_sb = smproj_pool.tile(
                    [N_BASIS, SBC], F32, tag="projT", name=f"projT{s}"
                )
                wT_sb = smproj_pool.tile(
                    [N_BASIS, SBC], BF16, tag="wT", name=f"wT{s}"
                )
                for g in range(SNG):
                    nc.scalar.copy(projT_sb[:, g * 512:(g + 1) * 512], proj_ps[g][:])
                for t in range(SNT):
                    pt = ps_t.tile([128, 128], F32, tag="t", name="pt")
                    nc.tensor.transpose(
                        pt[:, :N_BASIS],
                        projT_sb[:, t * 128:(t + 1) * 128],
                        id_sb[:N_BASIS, :N_BASIS],
                    )
                    negmax = sm_pool.tile([128, 1], F32, tag="negmax")
                    nc.vector.reduce_max(
                        negmax[:], pt[:, :N_BASIS], axis=X,
                        apply_absolute_value=True, negate=True,
                    )
                    absp = sm_pool.tile([128, N_BASIS], F32, tag="absp")
                    nc.scalar.activation(absp[:], pt[:, :N_BASIS], Act.Abs)
                    expv = sm_pool.tile([128, N_BASIS], F32, tag="expv")
                    sumexp = sm_pool.tile([128, 1], F32, tag="sumexp")
                    nc.scalar.activation(
                        expv[:], absp[:], Act.Exp, bias=negmax[:], scale=1.0,
                        accum_out=sumexp[:],
                    )
                    rsum = sm_pool.tile([128, 1], F32, tag="rsum")
                    nc.vector.reciprocal(rsum[:], sumexp[:])
                    wf = sm_pool.tile([128, N_BASIS], F32, tag="wf")
                    nc.vector.tensor_scalar_mul(wf[:], expv[:], rsum[:])
                    wt_ps = ps_t.tile([128, 128], F32, tag="t", name="wt")
                    nc.tensor.transpose(wt_ps[:N_BASIS, :], wf[:], id_sb[:])
                    nc.vector.tensor_copy(
                        wT_sb[:, t * 128:(t + 1) * 128], wt_ps[:N_BASIS, :]
                    )

                # -- phase C: attnT + residual, stream out --
                # epilogue alternates DVE tensor_tensor (psum+xt->sbuf) with a
                # PE identity-fold (+xt into psum) + ACT copy, so no single
                # engine is the bottleneck.
                for k in range(KC):
                    out_t = out_pool.tile([128, SBC], BF16, tag="out")
                    for g in range(SNG):
                        gs = slice(g * 512, (g + 1) * 512)
                        at_ps = ps_attn.tile([128, 512], F32, tag="at")
                        use_pe_fold = unit % 2 == 1
                        unit += 1
                        if use_pe_fold:
                            nc.tensor.matmul(
                                at_ps[:], idbf_sb[:], xt_tiles[(s, k)][:, gs],
                                start=True, stop=False,
                            )
                            nc.tensor.matmul(
                                at_ps[:],
                                bn_sb[:, k * 128:(k + 1) * 128],
                                wT_sb[:, gs],
                                start=False, stop=True,
                            )
                            nc.scalar.copy(out_t[:, gs], at_ps[:])
                        else:
                            nc.tensor.matmul(
                                at_ps[:],
                                bn_sb[:, k * 128:(k + 1) * 128],
                                wT_sb[:, gs],
                                start=True, stop=True,
                            )
                            nc.vector.tensor_tensor(
                                out_t[:, gs], at_ps[:],
                                xt_tiles[(s, k)][:, gs], op=add,
                            )
                    nc.sync.dma_start(
                        out_ext[k * 128:(k + 1) * 128, c0:c0 + SBC], out_t[:]
                    )

    return nc


def _split_multi_waits(bir: bytes) -> bytes:
    """This walrus build caps sync waits at ONE per instruction
    (CoreV3GenImpl setupSyncWait: 'Too many sync wait commands'), but Tile
    emits multi-sem waits. Hoist the extras onto wait-only EventSemaphore
    carriers placed just before the instruction on the same engine —
    program order makes the split semantically identical to the fused
    multi-wait."""
    import orjson

    m = orjson.loads(bir)
    n = 0
    for f in m["functions"]:
        for blk in f["blocks"]:
            insts = blk.get("instructions")
            if not insts:
                continue
            out = []
            changed = False
            for ins in insts:
                si = ins.get("sync_info")
                ow = (si or {}).get("on_wait") or []
                if len(ow) > 1:
                    changed = True
                    for w in ow[:-1]:
                        n += 1
                        out.append(
                            {
                                "debug": ins.get("debug"),
                                "engine": ins["engine"],
                                "ins": [],
                                "outs": [],
                                "name": f"waitsplit-{n}",
                                "opcode": "EventSemaphore",
                                "sync_info": {"on_update": [], "on_wait": [w]},
                            }
                        )
                    si["on_wait"] = [ow[-1]]
                out.append(ins)
            if changed:
                blk["instructions"] = out
    return orjson.dumps(m)


_NC_CACHE = {}


def _get_nc():
    if "nc" not in _NC_CACHE:
        nc = _build()
        orig_to_json = nc.to_json_bytes
        nc.to_json_bytes = lambda: _split_multi_waits(orig_to_json())
        _NC_CACHE["nc"] = nc
    return _NC_CACHE["nc"]


def kernel(x, _trace=False, _tmpdir=None):
    assert x.shape == (32, 512, 64, 64) and x.dtype == np.float32
    x_flat = np.ascontiguousarray(x.reshape(BC_TOTAL, L)).astype(NP_BF16)

    B64 = _blaschke_basis_f64(L)
    bn = np.ascontiguousarray(B64).astype(NP_BF16)          # [N, L]
    bt = np.ascontiguousarray(B64.T).astype(NP_BF16)        # [L, N]
    ident = np.eye(128, dtype=np.float32)

    in_maps = []
    for i in range(N_CORES):
        shard = x_flat[i * BC:(i + 1) * BC]                 # [BC, L] bf16
        xt = np.ascontiguousarray(shard.T)                  # [L, BC] bf16
        in_maps.append({"xt": xt, "bt": bt, "bn": bn, "ident": ident})

    nc = _get_nc()
    res = run_bass_kernel_spmd(
        nc, in_maps, core_ids=list(range(N_CORES)), trace=_trace, tmpdir=_tmpdir
    )

    outs = []
    for i in range(N_CORES):
        outT = np.asarray(res.results[i]["out"])            # [L, BC] bf16
        outs.append(np.ascontiguousarray(outT.T).astype(np.float32))
    out = np.concatenate(outs, axis=0).reshape(32, 512, 64, 64)
    if _trace:
        return out, res
    return out


# revision 11
# speedup vs baseline: 1.1962x; 1.1962x over previous
"""AFD channel attention on 8 TRN2 NeuronCores.

Math (per row r of x_flat [B*C, L], L = 64*64 = 4096, N = 64 basis fns):
    proj = x_flat @ B.T            [BC, N]
    w    = softmax(|proj|, -1)     [BC, N]
    out  = x_flat + w @ B          [BC, L]

Strategy: data-parallel over the 16384 (b, c) rows, 2048 rows per core.
Everything on-device runs in the TRANSPOSED domain (outT = xT + attnT) so
that the contraction dim of both matmuls lies on SBUF partitions with no
on-chip transpose of the big tensor: the host ships xT [L, 2048] per core
(bf16), the device returns outT [L, 2048] (bf16), host transposes back.

Perf structure (v2):
  - 2 BC-slices pipeline input DMA / softmax / output phases.
  - proj matmuls col-tiled in pairs (two L-chunks concurrently in PE cols
    0-63 / 64-127), attn matmuls row-tiled in pairs (two L-chunks in PE
    rows 0-63 / 64-127) — halves TensorE serial time.
  - residual epilogue split 3 ways: DVE psum-add, ACT copy + DVE bf16 4x
    add, ACT copy + GPSIMD add, so no single engine bottlenecks.
"""

import sys

for p in ("/opt/trn_rl_repo", "/root/.axon_site/_ro/trn_rl_repo"):
    if p not in sys.path:
        sys.path.append(p)

import numpy as np
import ml_dtypes

import concourse.bass as bass
import concourse.mybir as mybir
import concourse.tile as tile
from concourse.bass_utils import run_bass_kernel_spmd

BF16 = mybir.dt.bfloat16
F32 = mybir.dt.float32
NP_BF16 = ml_dtypes.bfloat16

N_BASIS = 64
R = 0.9
L = 4096            # 64 * 64
BC_TOTAL = 16384    # 32 * 512
N_CORES = 8
BC = BC_TOTAL // N_CORES   # 2048 rows per core
KC = L // 128       # 32 l-chunks of 128

NS = 2              # BC slices per core (phase pipelining)
SBC = BC // NS      # 1024 columns per slice
SNG = SBC // 512    # 2 groups of 512 per slice
SNT = SBC // 128    # 8 softmax tiles per slice

# epilogue path pattern: dve = single DVE psum+xt add; act_dve = ACT
# psum->sbuf copy + DVE bf16 4x add; act_gps = ACT copy + GPSIMD add.
EPI_PATTERN = (
    "dve", "act_dve", "dve", "act_gps", "dve", "act_dve",
    "dve", "act_gps", "dve", "act_dve", "dve", "act_dve",
)


def _blaschke_basis_f64(length):
    thetas = np.linspace(0.0, 2.0 * np.pi, N_BASIS, endpoint=False)
    t = np.linspace(0.0, 2.0 * np.pi, length)
    cosp = np.cos(t[None, :] - thetas[:, None])
    scale = np.sqrt(1.0 - R * R)
    return scale * (1.0 - R * cosp) / (1.0 - 2.0 * R * cosp + R * R)  # [N, L]


def _build():
    nc = bass.Bass()
    xt_ext = nc.declare_dram_parameter("xt", [L, BC], BF16, isOutput=False)
    bt_ext = nc.declare_dram_parameter("bt", [L, N_BASIS], BF16, isOutput=False)
    bn_ext = nc.declare_dram_parameter("bn", [N_BASIS, L], BF16, isOutput=False)
    id_ext = nc.declare_dram_parameter("ident", [128, 128], F32, isOutput=False)
    out_ext = nc.declare_dram_parameter("out", [L, BC], BF16, isOutput=True)

    add = mybir.AluOpType.add
    X = mybir.AxisListType.X
    Act = mybir.ActivationFunctionType

    with tile.TileContext(nc) as tc:
        with (
            tc.tile_pool(name="xt", bufs=NS * KC) as xt_pool,
            tc.tile_pool(name="const", bufs=1) as cpool,
            tc.tile_pool(name="smproj", bufs=2) as smproj_pool,
            tc.tile_pool(name="sm", bufs=4) as sm_pool,
            tc.tile_pool(name="attnsb", bufs=6) as attnsb_pool,
            tc.tile_pool(name="outp", bufs=4) as out_pool,
            tc.tile_pool(name="ps_proj", bufs=SNG, space="PSUM") as ps_proj,
            tc.tile_pool(name="ps_t", bufs=2, space="PSUM") as ps_t,
            tc.tile_pool(name="ps_attn", bufs=4, space="PSUM") as ps_attn,
        ):
            # -- constants --
            bt_sb = cpool.tile([128, KC * N_BASIS], BF16)   # chunk k at [:, 64k:64k+64]
            bn2_sb = cpool.tile([128, L], BF16)             # B duplicated on both halves
            id_sb = cpool.tile([128, 128], F32)
            nc.sync.dma_start(bn2_sb[:N_BASIS, :], bn_ext[:])
            nc.sync.dma_start(bn2_sb[N_BASIS:, :], bn_ext[:])
            nc.sync.dma_start(id_sb[:], id_ext[:])
            for k in range(KC):
                nc.sync.dma_start(
                    bt_sb[:, k * N_BASIS:(k + 1) * N_BASIS],
                    bt_ext[k * 128:(k + 1) * 128, :],
                )

            # -- all input DMAs issued up-front so the SP HWDGE stream never
            #    stalls behind output DMAs of an earlier slice --
            xt_tiles = {}
            for s in range(NS):
                for k in range(KC):
                    xt_t = xt_pool.tile([128, SBC], BF16, tag="xt", name=f"xt{s}_{k}")
                    nc.sync.dma_start(
                        xt_t[:],
                        xt_ext[k * 128:(k + 1) * 128, s * SBC:(s + 1) * SBC],
                    )
                    xt_tiles[(s, k)] = xt_t

            proj_ps_s = {}

            def emit_proj(s):
                # col-tiled pairs: chunk 2k -> PE cols 0-63 (psum rows 0:64),
                # chunk 2k+1 -> PE cols 64-127 (psum rows 64:128)
                proj_ps = [
                    ps_proj.tile([128, 512], F32, tag="proj", name=f"proj{s}_{g}")
                    for g in range(SNG)
                ]
                for kp in range(KC // 2):
                    ka, kb = 2 * kp, 2 * kp + 1
                    for g in range(SNG):
                        gs = slice(g * 512, (g + 1) * 512)
                        nc.tensor.matmul(
                            proj_ps[g][:N_BASIS, :],
                            bt_sb[:, ka * N_BASIS:(ka + 1) * N_BASIS],
                            xt_tiles[(s, ka)][:, gs],
                            start=(kp == 0), stop=(kp == KC // 2 - 1),
                            tile_position=(0, 0),
                        )
                        nc.tensor.matmul(
                            proj_ps[g][N_BASIS:, :],
                            bt_sb[:, kb * N_BASIS:(kb + 1) * N_BASIS],
                            xt_tiles[(s, kb)][:, gs],
                            start=(kp == 0), stop=(kp == KC // 2 - 1),
                            tile_position=(0, N_BASIS),
                        )
                proj_ps_s[s] = proj_ps

            def emit_softmax(s):
                proj_ps = proj_ps_s[s]
                projT_sb = smproj_pool.tile(
                    [N_BASIS, SBC], F32, tag="projT", name=f"projT{s}"
                )
                half_sb = smproj_pool.tile(
                    [N_BASIS, 512], F32, tag="half", name=f"half{s}"
                )
                wT2_sb = smproj_pool.tile(
                    [128, SBC], BF16, tag="wT", name=f"wT{s}"
                )
                for g in range(SNG):
                    gs = slice(g * 512, (g + 1) * 512)
                    nc.scalar.copy(half_sb[:], proj_ps[g][:N_BASIS, :])
                    nc.vector.tensor_tensor(
                        projT_sb[:, gs], proj_ps[g][N_BASIS:, :], half_sb[:], op=add
                    )
                for t in range(SNT):
                    ts = slice(t * 128, (t + 1) * 128)
                    pt = ps_t.tile([128, 128], F32, tag="t", name="pt")
                    nc.tensor.transpose(
                        pt[:, :N_BASIS], projT_sb[:, ts], id_sb[:N_BASIS, :N_BASIS]
                    )
                    negmax = sm_pool.tile([128, 1], F32, tag="negmax")
                    nc.vector.reduce_max(
                        negmax[:], pt[:, :N_BASIS], axis=X,
                        apply_absolute_value=True, negate=True,
                    )
                    absp = sm_pool.tile([128, N_BASIS], F32, tag="absp")
                    nc.scalar.activation(absp[:], pt[:, :N_BASIS], Act.Abs)
                    expv = sm_pool.tile([128, N_BASIS], F32, tag="expv")
                    sumexp = sm_pool.tile([128, 1], F32, tag="sumexp")
                    nc.scalar.activation(
                        expv[:], absp[:], Act.Exp, bias=negmax[:], scale=1.0,
                        accum_out=sumexp[:],
                    )
                    rsum = sm_pool.tile([128, 1], F32, tag="rsum")
                    nc.vector.reciprocal(rsum[:], sumexp[:])
                    wf = sm_pool.tile([128, N_BASIS], F32, tag="wf")
                    nc.vector.tensor_scalar_mul(wf[:], expv[:], rsum[:])
                    wt_ps = ps_t.tile([128, 128], F32, tag="t", name="wt")
                    nc.tensor.transpose(wt_ps[:N_BASIS, :], wf[:], id_sb[:])
                    nc.scalar.copy(wT2_sb[:N_BASIS, ts], wt_ps[:N_BASIS, :])
                    nc.vector.tensor_copy(wT2_sb[N_BASIS:, ts], wt_ps[:N_BASIS, :])
                return wT2_sb

            epi = [0]

            def emit_attn(s, wT2_sb):
                c0 = s * SBC
                # row-tiled pairs: chunk 2k uses PE rows 0-63, chunk 2k+1
                # rows 64-127 (B and wT duplicated on both partition halves)
                for kp in range(KC // 2):
                    out_a = out_pool.tile([128, SBC], BF16, tag="out", name="outa")
                    out_b = out_pool.tile([128, SBC], BF16, tag="out", name="outb")
                    for g in range(SNG):
                        gs = slice(g * 512, (g + 1) * 512)
                        ka, kb = 2 * kp, 2 * kp + 1
                        ps_a = ps_attn.tile([128, 512], F32, tag="at", name="psa")
                        ps_b = ps_attn.tile([128, 512], F32, tag="at", name="psb")
                        nc.tensor.matmul(
                            ps_a[:],
                            bn2_sb[:N_BASIS, ka * 128:(ka + 1) * 128],
                            wT2_sb[:N_BASIS, gs],
                            start=True, stop=True,
                            tile_position=(0, 0),
                        )
                        nc.tensor.matmul(
                            ps_b[:],
                            bn2_sb[N_BASIS:, kb * 128:(kb + 1) * 128],
                            wT2_sb[N_BASIS:, gs],
                            start=True, stop=True,
                            tile_position=(N_BASIS, 0),
                        )
                        for out_t, k, at_ps in ((out_a, ka, ps_a), (out_b, kb, ps_b)):
                            path = EPI_PATTERN[epi[0] % len(EPI_PATTERN)]
                            epi[0] += 1
                            xt_sl = xt_tiles[(s, k)][:, gs]
                            if path == "dve":
                                nc.vector.tensor_tensor(
                                    out_t[:, gs], at_ps[:], xt_sl, op=add
                                )
                            else:
                                attn_sb = attnsb_pool.tile(
                                    [128, 512], BF16, tag="attnsb"
                                )
                                nc.scalar.copy(attn_sb[:], at_ps[:])
                                eng = nc.vector if path == "act_dve" else nc.gpsimd
                                eng.tensor_tensor(
                                    out_t[:, gs], attn_sb[:], xt_sl, op=add
                                )
                    nc.sync.dma_start(
                        out_ext[2 * kp * 128:(2 * kp + 1) * 128, c0:c0 + SBC],
                        out_a[:],
                    )
                    nc.sync.dma_start(
                        out_ext[(2 * kp + 1) * 128:(2 * kp + 2) * 128, c0:c0 + SBC],
                        out_b[:],
                    )

            # slice-pipelined emission: proj(s1) sits between B(s0) and C(s0)
            # in the PE stream so PE stays busy while softmax(s0) cooks.
            emit_proj(0)
            wt0 = emit_softmax(0)
            emit_proj(1)
            emit_attn(0, wt0)
            wt1 = emit_softmax(1)
            emit_attn(1, wt1)

    return nc


def _split_multi_waits(bir: bytes) -> bytes:
    """This walrus build caps sync waits at ONE per instruction
    (CoreV3GenImpl setupSyncWait: 'Too many sync wait commands'), but Tile
    emits multi-sem waits. Hoist the extras onto wait-only EventSemaphore
    carriers placed just before the instruction on the same engine —
    program order makes the split semantically identical to the fused
    multi-wait."""
    import orjson

    m = orjson.loads(bir)
    n = 0
    for f in m["functions"]:
        for blk in f["blocks"]:
            insts = blk.get("instructions")
            if not insts:
                continue
            out = []
            changed = False
            for ins in insts:
                si = ins.get("sync_info")
                ow = (si or {}).get("on_wait") or []
                if len(ow) > 1:
                    changed = True
                    for w in ow[:-1]:
                        n += 1
                        out.append(
                            {
                                "debug": ins.get("debug"),
                                "engine": ins["engine"],
                                "ins": [],
                                "outs": [],
                                "name": f"waitsplit-{n}",
                                "opcode": "EventSemaphore",
                                "sync_info": {"on_update": [], "on_wait": [w]},
                            }
                        )
                    si["on_wait"] = [ow[-1]]
                out.append(ins)
            if changed:
                blk["instructions"] = out
    return orjson.dumps(m)


_NC_CACHE = {}


def _get_nc():
    if "nc" not in _NC_CACHE:
        nc = _build()
        orig_to_json = nc.to_json_bytes
        nc.to_json_bytes = lambda: _split_multi_waits(orig_to_json())
        _NC_CACHE["nc"] = nc
    return _NC_CACHE["nc"]


def kernel(x, _trace=False, _tmpdir=None):
    assert x.shape == (32, 512, 64, 64) and x.dtype == np.float32
    x_flat = np.ascontiguousarray(x.reshape(BC_TOTAL, L)).astype(NP_BF16)

    B64 = _blaschke_basis_f64(L)
    bn = np.ascontiguousarray(B64).astype(NP_BF16)          # [N, L]
    bt = np.ascontiguousarray(B64.T).astype(NP_BF16)        # [L, N]
    ident = np.eye(128, dtype=np.float32)

    in_maps = []
    for i in range(N_CORES):
        shard = x_flat[i * BC:(i + 1) * BC]                 # [BC, L] bf16
        xt = np.ascontiguousarray(shard.T)                  # [L, BC] bf16
        in_maps.append({"xt": xt, "bt": bt, "bn": bn, "ident": ident})

    nc = _get_nc()
    res = run_bass_kernel_spmd(
        nc, in_maps, core_ids=list(range(N_CORES)), trace=_trace, tmpdir=_tmpdir
    )

    outs = []
    for i in range(N_CORES):
        outT = np.asarray(res.results[i]["out"])            # [L, BC] bf16
        outs.append(np.ascontiguousarray(outT.T).astype(np.float32))
    out = np.concatenate(outs, axis=0).reshape(32, 512, 64, 64)
    if _trace:
        return out, res
    return out


# revision 12
# speedup vs baseline: 1.5478x; 1.2939x over previous
"""AFD channel attention on 8 TRN2 NeuronCores.

Math (per row r of x_flat [B*C, L], L = 64*64 = 4096, N = 64 basis fns):
    proj = x_flat @ B.T            [BC, N]
    w    = softmax(|proj|, -1)     [BC, N]
    out  = x_flat + w @ B          [BC, L]

Strategy: data-parallel over the 16384 (b, c) rows, 2048 rows per core.
Everything on-device runs in the TRANSPOSED domain (outT = xT + attnT) so
that the contraction dim of both matmuls lies on SBUF partitions with no
on-chip transpose of the big tensor: the host ships xT [L, 2048] per core
(bf16), the device returns outT [L, 2048] (bf16), host transposes back.

Perf structure (v3):
  - 2 BC-slices pipeline input DMA / softmax / output phases; all input
    DMAs are issued before any output DMA so the SP HWDGE stream never
    stalls behind compute.
  - DMAs move two 128-row l-chunks per instruction (3D access pattern)
    to halve sequencer issue cost.
  - residual epilogue split 3 ways: DVE psum-add, ACT copy + DVE bf16
    add, ACT copy + GPSIMD add, so no single engine bottlenecks.
  - softmax scale+cast fused into ACT (per-partition scale operand).
"""

import sys

for p in ("/opt/trn_rl_repo", "/root/.axon_site/_ro/trn_rl_repo"):
    if p not in sys.path:
        sys.path.append(p)

import numpy as np
import ml_dtypes

import concourse.bass as bass
import concourse.mybir as mybir
import concourse.tile as tile
from concourse.bass_utils import run_bass_kernel_spmd

BF16 = mybir.dt.bfloat16
F32 = mybir.dt.float32
NP_BF16 = ml_dtypes.bfloat16

N_BASIS = 64
R = 0.9
L = 4096            # 64 * 64
BC_TOTAL = 16384    # 32 * 512
N_CORES = 8
BC = BC_TOTAL // N_CORES   # 2048 rows per core
KC = L // 128       # 32 l-chunks of 128
KP = KC // 2        # 16 chunk-pairs (DMA granularity)

NS = 2              # BC slices per core (phase pipelining)
SBC = BC // NS      # 1024 columns per slice
SNG = SBC // 512    # 2 groups of 512 per slice
SNT = SBC // 128    # 8 softmax tiles per slice

# epilogue path mix per 16 units: DVE direct / ACT+DVE / ACT+GPSIMD
EPI_PATTERN = (
    "dve", "act_dve", "dve", "act_gps", "dve", "act_dve", "dve", "dve",
    "act_dve", "dve", "act_gps", "dve", "act_dve", "dve", "act_gps", "dve",
)


def _blaschke_basis_f64(length):
    thetas = np.linspace(0.0, 2.0 * np.pi, N_BASIS, endpoint=False)
    t = np.linspace(0.0, 2.0 * np.pi, length)
    cosp = np.cos(t[None, :] - thetas[:, None])
    scale = np.sqrt(1.0 - R * R)
    return scale * (1.0 - R * cosp) / (1.0 - 2.0 * R * cosp + R * R)  # [N, L]


def _build():
    nc = bass.Bass()
    xt_ext = nc.declare_dram_parameter("xt", [L, BC], BF16, isOutput=False)
    bt_ext = nc.declare_dram_parameter("bt", [L, N_BASIS], BF16, isOutput=False)
    bn_ext = nc.declare_dram_parameter("bn", [N_BASIS, L], BF16, isOutput=False)
    id_ext = nc.declare_dram_parameter("ident", [128, 128], F32, isOutput=False)
    out_ext = nc.declare_dram_parameter("out", [L, BC], BF16, isOutput=True)

    # [L, BC] viewed as [KP, p, j, c]: dram row = 256*kp + 128*j + p
    xt_v = xt_ext[:].rearrange("(a j p) c -> a p j c", p=128, j=2)
    out_v = out_ext[:].rearrange("(a j p) c -> a p j c", p=128, j=2)

    add = mybir.AluOpType.add
    X = mybir.AxisListType.X
    Act = mybir.ActivationFunctionType

    with tile.TileContext(nc) as tc:
        with (
            tc.tile_pool(name="xt", bufs=NS * KP) as xt_pool,
            tc.tile_pool(name="const", bufs=1) as cpool,
            tc.tile_pool(name="smproj", bufs=2) as smproj_pool,
            tc.tile_pool(name="sm", bufs=4) as sm_pool,
            tc.tile_pool(name="attnsb", bufs=6) as attnsb_pool,
            tc.tile_pool(name="outp", bufs=4) as out_pool,
            tc.tile_pool(name="ps_proj", bufs=SNG, space="PSUM") as ps_proj,
            tc.tile_pool(name="ps_t", bufs=1, space="PSUM") as ps_t,
            tc.tile_pool(name="ps_attn", bufs=4, space="PSUM") as ps_attn,
        ):
            # -- constants --
            bt_sb = cpool.tile([128, KC * N_BASIS], BF16)   # chunk k at [:, 64k:64k+64]
            bn_sb = cpool.tile([N_BASIS, L], BF16)
            id_sb = cpool.tile([128, 128], F32)
            idbf_sb = cpool.tile([128, 128], BF16)
            nc.sync.dma_start(bn_sb[:], bn_ext[:])
            nc.sync.dma_start(id_sb[:], id_ext[:])
            nc.vector.tensor_copy(idbf_sb[:], id_sb[:])
            for k in range(KC):
                nc.sync.dma_start(
                    bt_sb[:, k * N_BASIS:(k + 1) * N_BASIS],
                    bt_ext[k * 128:(k + 1) * 128, :],
                )

            # -- all input DMAs issued up-front; one DMA per chunk-PAIR --
            # tile cols [0:SBC] = chunk 2kp, [SBC:2*SBC] = chunk 2kp+1
            xt_tiles = {}
            for s in range(NS):
                cs = slice(s * SBC, (s + 1) * SBC)
                for kp in range(KP):
                    xt_t = xt_pool.tile(
                        [128, 2 * SBC], BF16, tag="xt", name=f"xt{s}_{kp}"
                    )
                    nc.sync.dma_start(
                        xt_t[:].rearrange("p (j c) -> p j c", j=2),
                        xt_v[kp, :, :, cs],
                    )
                    xt_tiles[(s, kp)] = xt_t

            def xchunk(s, k, gs):
                half = (k % 2) * SBC
                return xt_tiles[(s, k // 2)][:, half + gs.start:half + gs.stop]

            proj_ps_s = {}

            def emit_proj(s):
                proj_ps = [
                    ps_proj.tile([N_BASIS, 512], F32, tag="proj", name=f"proj{s}_{g}")
                    for g in range(SNG)
                ]
                for k in range(KC):
                    for g in range(SNG):
                        gs = slice(g * 512, (g + 1) * 512)
                        nc.tensor.matmul(
                            proj_ps[g][:],
                            bt_sb[:, k * N_BASIS:(k + 1) * N_BASIS],
                            xchunk(s, k, gs),
                            start=(k == 0), stop=(k == KC - 1),
                        )
                proj_ps_s[s] = proj_ps

            def emit_softmax(s):
                proj_ps = proj_ps_s[s]
                projT_sb = smproj_pool.tile(
                    [N_BASIS, SBC], F32, tag="projT", name=f"projT{s}"
                )
                wT_sb = smproj_pool.tile(
                    [N_BASIS, SBC], BF16, tag="wT", name=f"wT{s}"
                )
                for g in range(SNG):
                    gs = slice(g * 512, (g + 1) * 512)
                    nc.scalar.copy(projT_sb[:, gs], proj_ps[g][:])
                for t in range(SNT):
                    ts = slice(t * 128, (t + 1) * 128)
                    pt = ps_t.tile([128, N_BASIS], F32, tag="pt", name="pt")
                    nc.tensor.transpose(
                        pt[:], projT_sb[:, ts], id_sb[:N_BASIS, :N_BASIS]
                    )
                    negmax = sm_pool.tile([128, 1], F32, tag="negmax")
                    nc.vector.reduce_max(
                        negmax[:], pt[:], axis=X,
                        apply_absolute_value=True, negate=True,
                    )
                    absp = sm_pool.tile([128, N_BASIS], F32, tag="absp")
                    nc.scalar.activation(absp[:], pt[:], Act.Abs)
                    expv = sm_pool.tile([128, N_BASIS], F32, tag="expv")
                    sumexp = sm_pool.tile([128, 1], F32, tag="sumexp")
                    nc.scalar.activation(
                        expv[:], absp[:], Act.Exp, bias=negmax[:], scale=1.0,
                        accum_out=sumexp[:],
                    )
                    rsum = sm_pool.tile([128, 1], F32, tag="rsum")
                    nc.vector.reciprocal(rsum[:], sumexp[:])
                    wfb = sm_pool.tile([128, N_BASIS], BF16, tag="wfb")
                    nc.scalar.mul(wfb[:], expv[:], rsum[:])
                    wt_ps = ps_t.tile([N_BASIS, 128], BF16, tag="wt", name="wt")
                    nc.tensor.transpose(wt_ps[:], wfb[:], idbf_sb[:])
                    nc.scalar.copy(wT_sb[:, ts], wt_ps[:])
                return wT_sb

            epi = [0]

            def emit_attn(s, wT_sb):
                cs = slice(s * SBC, (s + 1) * SBC)
                for kp in range(KP):
                    out_t = out_pool.tile([128, 2 * SBC], BF16, tag="out")
                    for j in range(2):
                        k = 2 * kp + j
                        for g in range(SNG):
                            gs = slice(g * 512, (g + 1) * 512)
                            at_ps = ps_attn.tile([128, 512], F32, tag="at")
                            nc.tensor.matmul(
                                at_ps[:],
                                bn_sb[:, k * 128:(k + 1) * 128],
                                wT_sb[:, gs],
                                start=True, stop=True,
                            )
                            o_sl = out_t[:, j * SBC + gs.start:j * SBC + gs.stop]
                            path = EPI_PATTERN[epi[0] % len(EPI_PATTERN)]
                            epi[0] += 1
                            if path == "dve":
                                nc.vector.tensor_tensor(
                                    o_sl, at_ps[:], xchunk(s, k, gs), op=add
                                )
                            else:
                                attn_sb = attnsb_pool.tile(
                                    [128, 512], BF16, tag="attnsb"
                                )
                                nc.scalar.copy(attn_sb[:], at_ps[:])
                                eng = nc.vector if path == "act_dve" else nc.gpsimd
                                eng.tensor_tensor(
                                    o_sl, attn_sb[:], xchunk(s, k, gs), op=add
                                )
                    nc.sync.dma_start(
                        out_v[kp, :, :, cs],
                        out_t[:].rearrange("p (j c) -> p j c", j=2),
                    )

            # slice-pipelined emission: proj(s1) sits between B(s0) and C(s0)
            # in the PE stream so PE stays busy while softmax(s0) cooks.
            emit_proj(0)
            wt0 = emit_softmax(0)
            emit_proj(1)
            emit_attn(0, wt0)
            wt1 = emit_softmax(1)
            emit_attn(1, wt1)

    return nc


def _split_multi_waits(bir: bytes) -> bytes:
    """This walrus build caps sync waits at ONE per instruction
    (CoreV3GenImpl setupSyncWait: 'Too many sync wait commands'), but Tile
    emits multi-sem waits. Hoist the extras onto wait-only EventSemaphore
    carriers placed just before the instruction on the same engine —
    program order makes the split semantically identical to the fused
    multi-wait."""
    import orjson

    m = orjson.loads(bir)
    n = 0
    for f in m["functions"]:
        for blk in f["blocks"]:
            insts = blk.get("instructions")
            if not insts:
                continue
            out = []
            changed = False
            for ins in insts:
                si = ins.get("sync_info")
                ow = (si or {}).get("on_wait") or []
                if len(ow) > 1:
                    changed = True
                    for w in ow[:-1]:
                        n += 1
                        out.append(
                            {
                                "debug": ins.get("debug"),
                                "engine": ins["engine"],
                                "ins": [],
                                "outs": [],
                                "name": f"waitsplit-{n}",
                                "opcode": "EventSemaphore",
                                "sync_info": {"on_update": [], "on_wait": [w]},
                            }
                        )
                    si["on_wait"] = [ow[-1]]
                out.append(ins)
            if changed:
                blk["instructions"] = out
    return orjson.dumps(m)


_NC_CACHE = {}


def _get_nc():
    if "nc" not in _NC_CACHE:
        nc = _build()
        orig_to_json = nc.to_json_bytes
        nc.to_json_bytes = lambda: _split_multi_waits(orig_to_json())
        _NC_CACHE["nc"] = nc
    return _NC_CACHE["nc"]


def kernel(x, _trace=False, _tmpdir=None):
    assert x.shape == (32, 512, 64, 64) and x.dtype == np.float32
    x_flat = np.ascontiguousarray(x.reshape(BC_TOTAL, L)).astype(NP_BF16)

    B64 = _blaschke_basis_f64(L)
    bn = np.ascontiguousarray(B64).astype(NP_BF16)          # [N, L]
    bt = np.ascontiguousarray(B64.T).astype(NP_BF16)        # [L, N]
    ident = np.eye(128, dtype=np.float32)

    in_maps = []
    for i in range(N_CORES):
        shard = x_flat[i * BC:(i + 1) * BC]                 # [BC, L] bf16
        xt = np.ascontiguousarray(shard.T)                  # [L, BC] bf16
        in_maps.append({"xt": xt, "bt": bt, "bn": bn, "ident": ident})

    nc = _get_nc()
    res = run_bass_kernel_spmd(
        nc, in_maps, core_ids=list(range(N_CORES)), trace=_trace, tmpdir=_tmpdir
    )

    outs = []
    for i in range(N_CORES):
        outT = np.asarray(res.results[i]["out"])            # [L, BC] bf16
        outs.append(np.ascontiguousarray(outT.T).astype(np.float32))
    out = np.concatenate(outs, axis=0).reshape(32, 512, 64, 64)
    if _trace:
        return out, res
    return out


# revision 16
# speedup vs baseline: 1.7214x; 1.1122x over previous
"""AFD channel attention on 8 TRN2 NeuronCores.

Math (per row r of x_flat [B*C, L], L = 64*64 = 4096, N = 64 basis fns):
    proj = x_flat @ B.T            [BC, N]
    w    = softmax(|proj|, -1)     [BC, N]
    out  = x_flat + w @ B          [BC, L]

Strategy: data-parallel over the 16384 (b, c) rows, 2048 rows per core.
Everything on-device runs in the TRANSPOSED domain (outT = xT + attnT) so
that the contraction dim of both matmuls lies on SBUF partitions with no
on-chip transpose of the big tensor: the host ships xT [L, 2048] per core
(bf16), the device returns outT [L, 2048] (bf16), host transposes back.

Perf structure (v3):
  - 2 BC-slices pipeline input DMA / softmax / output phases; all input
    DMAs are issued before any output DMA so the SP HWDGE stream never
    stalls behind compute.
  - DMAs move two 128-row l-chunks per instruction (3D access pattern)
    to halve sequencer issue cost.
  - residual epilogue split 3 ways: DVE psum-add, ACT copy + DVE bf16
    add, ACT copy + GPSIMD add, so no single engine bottlenecks.
  - softmax scale+cast fused into ACT (per-partition scale operand).
"""

import sys

for p in ("/opt/trn_rl_repo", "/root/.axon_site/_ro/trn_rl_repo"):
    if p not in sys.path:
        sys.path.append(p)

import numpy as np
import ml_dtypes

import concourse.bass as bass
import concourse.mybir as mybir
import concourse.tile as tile
from concourse.bass_utils import run_bass_kernel_spmd

BF16 = mybir.dt.bfloat16
F32 = mybir.dt.float32
NP_BF16 = ml_dtypes.bfloat16

N_BASIS = 64
R = 0.9
L = 4096            # 64 * 64
BC_TOTAL = 16384    # 32 * 512
N_CORES = 8
BC = BC_TOTAL // N_CORES   # 2048 rows per core
KC = L // 128       # 32 l-chunks of 128
KP = KC // 2        # 16 chunk-pairs (DMA granularity)

NS = 2              # BC slices per core (phase pipelining)
SBC = BC // NS      # 1024 columns per slice
SNG = SBC // 512    # 2 groups of 512 per slice
SNT = SBC // 128    # 8 softmax tiles per slice

# epilogue path mix per 16 units: DVE direct / ACT+DVE / ACT+GPSIMD
EPI_PATTERN = (
    "dve", "act_dve", "dve", "act_gps", "dve", "act_dve", "dve", "dve",
    "act_dve", "dve", "act_gps", "dve", "act_dve", "dve", "act_gps", "dve",
)


def _blaschke_basis_f64(length):
    thetas = np.linspace(0.0, 2.0 * np.pi, N_BASIS, endpoint=False)
    t = np.linspace(0.0, 2.0 * np.pi, length)
    cosp = np.cos(t[None, :] - thetas[:, None])
    scale = np.sqrt(1.0 - R * R)
    return scale * (1.0 - R * cosp) / (1.0 - 2.0 * R * cosp + R * R)  # [N, L]


def _build():
    nc = bass.Bass()
    xt_ext = nc.declare_dram_parameter("xt", [L, BC], BF16, isOutput=False)
    # bt ships host-packed as [128, KC*64]: bt[p, 64k+n] = B[n, 128k+p],
    # so one contiguous DMA loads every proj lhsT chunk.
    bt_ext = nc.declare_dram_parameter("bt", [128, KC * N_BASIS], BF16, isOutput=False)
    bn_ext = nc.declare_dram_parameter("bn", [N_BASIS, L], BF16, isOutput=False)
    id_ext = nc.declare_dram_parameter("ident", [128, 128], F32, isOutput=False)
    out_ext = nc.declare_dram_parameter("out", [L, BC], BF16, isOutput=True)

    # [L, BC] viewed as [KP, p, j, c]: dram row = 256*kp + 128*j + p
    xt_v = xt_ext[:].rearrange("(a j p) c -> a p j c", p=128, j=2)
    out_v = out_ext[:].rearrange("(a j p) c -> a p j c", p=128, j=2)

    add = mybir.AluOpType.add
    X = mybir.AxisListType.X
    Act = mybir.ActivationFunctionType

    with tile.TileContext(nc) as tc:
        with (
            tc.tile_pool(name="xt", bufs=NS * KP) as xt_pool,
            tc.tile_pool(name="const", bufs=1) as cpool,
            tc.tile_pool(name="smproj", bufs=2) as smproj_pool,
            tc.tile_pool(name="sm", bufs=4) as sm_pool,
            tc.tile_pool(name="attnsb", bufs=6) as attnsb_pool,
            tc.tile_pool(name="outp", bufs=4) as out_pool,
            tc.tile_pool(name="ps_proj", bufs=SNG, space="PSUM") as ps_proj,
            tc.tile_pool(name="ps_t", bufs=1, space="PSUM") as ps_t,
            tc.tile_pool(name="ps_attn", bufs=4, space="PSUM") as ps_attn,
        ):
            # -- constants --
            bt_sb = cpool.tile([128, KC * N_BASIS], BF16)   # chunk k at [:, 64k:64k+64]
            bn_sb = cpool.tile([N_BASIS, L], BF16)
            id_sb = cpool.tile([128, 128], F32)
            idbf_sb = cpool.tile([128, 128], BF16)
            nc.sync.dma_start(bn_sb[:], bn_ext[:])
            nc.sync.dma_start(id_sb[:], id_ext[:])
            nc.sync.dma_start(bt_sb[:], bt_ext[:])
            nc.vector.tensor_copy(idbf_sb[:], id_sb[:])

            # -- all input DMAs issued up-front; one DMA per chunk-PAIR --
            # tile cols [0:SBC] = chunk 2kp, [SBC:2*SBC] = chunk 2kp+1
            xt_tiles = {}
            for s in range(NS):
                cs = slice(s * SBC, (s + 1) * SBC)
                for kp in range(KP):
                    xt_t = xt_pool.tile(
                        [128, 2 * SBC], BF16, tag="xt", name=f"xt{s}_{kp}"
                    )
                    nc.sync.dma_start(
                        xt_t[:].rearrange("p (j c) -> p j c", j=2),
                        xt_v[kp, :, :, cs],
                    )
                    xt_tiles[(s, kp)] = xt_t

            def xchunk(s, k, gs):
                half = (k % 2) * SBC
                return xt_tiles[(s, k // 2)][:, half + gs.start:half + gs.stop]

            proj_ps_s = {}

            def emit_proj(s):
                proj_ps = [
                    ps_proj.tile([N_BASIS, 512], F32, tag="proj", name=f"proj{s}_{g}")
                    for g in range(SNG)
                ]
                for k in range(KC):
                    for g in range(SNG):
                        gs = slice(g * 512, (g + 1) * 512)
                        nc.tensor.matmul(
                            proj_ps[g][:],
                            bt_sb[:, k * N_BASIS:(k + 1) * N_BASIS],
                            xchunk(s, k, gs),
                            start=(k == 0), stop=(k == KC - 1),
                        )
                proj_ps_s[s] = proj_ps

            def emit_softmax(s):
                proj_ps = proj_ps_s[s]
                projT_sb = smproj_pool.tile(
                    [N_BASIS, SBC], F32, tag="projT", name=f"projT{s}"
                )
                wT_sb = smproj_pool.tile(
                    [N_BASIS, SBC], BF16, tag="wT", name=f"wT{s}"
                )
                for g in range(SNG):
                    gs = slice(g * 512, (g + 1) * 512)
                    nc.scalar.copy(projT_sb[:, gs], proj_ps[g][:])
                for t in range(SNT):
                    ts = slice(t * 128, (t + 1) * 128)
                    pt = ps_t.tile([128, N_BASIS], F32, tag="pt", name="pt")
                    nc.tensor.transpose(
                        pt[:], projT_sb[:, ts], id_sb[:N_BASIS, :N_BASIS]
                    )
                    negmax = sm_pool.tile([128, 1], F32, tag="negmax")
                    nc.vector.reduce_max(
                        negmax[:], pt[:], axis=X,
                        apply_absolute_value=True, negate=True,
                    )
                    absp = sm_pool.tile([128, N_BASIS], F32, tag="absp")
                    nc.scalar.activation(absp[:], pt[:], Act.Abs)
                    expv = sm_pool.tile([128, N_BASIS], F32, tag="expv")
                    sumexp = sm_pool.tile([128, 1], F32, tag="sumexp")
                    nc.scalar.activation(
                        expv[:], absp[:], Act.Exp, bias=negmax[:], scale=1.0,
                        accum_out=sumexp[:],
                    )
                    rsum = sm_pool.tile([128, 1], F32, tag="rsum")
                    nc.vector.reciprocal(rsum[:], sumexp[:])
                    wfb = sm_pool.tile([128, N_BASIS], BF16, tag="wfb")
                    nc.scalar.mul(wfb[:], expv[:], rsum[:])
                    wt_ps = ps_t.tile([N_BASIS, 128], BF16, tag="wt", name="wt")
                    nc.tensor.transpose(wt_ps[:], wfb[:], idbf_sb[:])
                    nc.vector.tensor_copy(wT_sb[:, ts], wt_ps[:])
                return wT_sb

            epi = [0]

            def emit_attn(s, wT_sb):
                cs = slice(s * SBC, (s + 1) * SBC)
                for kp in range(KP):
                    out_t = out_pool.tile([128, 2 * SBC], BF16, tag="out")
                    for j in range(2):
                        k = 2 * kp + j
                        for g in range(SNG):
                            gs = slice(g * 512, (g + 1) * 512)
                            at_ps = ps_attn.tile([128, 512], F32, tag="at")
                            nc.tensor.matmul(
                                at_ps[:],
                                bn_sb[:, k * 128:(k + 1) * 128],
                                wT_sb[:, gs],
                                start=True, stop=True,
                            )
                            o_sl = out_t[:, j * SBC + gs.start:j * SBC + gs.stop]
                            path = EPI_PATTERN[epi[0] % len(EPI_PATTERN)]
                            epi[0] += 1
                            if path == "dve":
                                nc.vector.tensor_tensor(
                                    o_sl, at_ps[:], xchunk(s, k, gs), op=add
                                )
                            else:
                                attn_sb = attnsb_pool.tile(
                                    [128, 512], BF16, tag="attnsb"
                                )
                                nc.scalar.copy(attn_sb[:], at_ps[:])
                                eng = nc.vector if path == "act_dve" else nc.gpsimd
                                eng.tensor_tensor(
                                    o_sl, attn_sb[:], xchunk(s, k, gs), op=add
                                )
                    nc.sync.dma_start(
                        out_v[kp, :, :, cs],
                        out_t[:].rearrange("p (j c) -> p j c", j=2),
                    )

            # slice-pipelined emission: proj(s1) sits between B(s0) and C(s0)
            # in the PE stream so PE stays busy while softmax(s0) cooks.
            emit_proj(0)
            wt0 = emit_softmax(0)
            emit_proj(1)
            emit_attn(0, wt0)
            wt1 = emit_softmax(1)
            emit_attn(1, wt1)

    return nc


def _split_multi_waits(bir: bytes) -> bytes:
    """This walrus build caps sync waits at ONE per instruction
    (CoreV3GenImpl setupSyncWait: 'Too many sync wait commands'), but Tile
    emits multi-sem waits. Hoist the extras onto wait-only EventSemaphore
    carriers placed just before the instruction on the same engine —
    program order makes the split semantically identical to the fused
    multi-wait."""
    import orjson

    m = orjson.loads(bir)
    n = 0
    for f in m["functions"]:
        for blk in f["blocks"]:
            insts = blk.get("instructions")
            if not insts:
                continue
            out = []
            changed = False
            for ins in insts:
                si = ins.get("sync_info")
                ow = (si or {}).get("on_wait") or []
                if len(ow) > 1:
                    changed = True
                    for w in ow[:-1]:
                        n += 1
                        out.append(
                            {
                                "debug": ins.get("debug"),
                                "engine": ins["engine"],
                                "ins": [],
                                "outs": [],
                                "name": f"waitsplit-{n}",
                                "opcode": "EventSemaphore",
                                "sync_info": {"on_update": [], "on_wait": [w]},
                            }
                        )
                    si["on_wait"] = [ow[-1]]
                out.append(ins)
            if changed:
                blk["instructions"] = out
    return orjson.dumps(m)


_NC_CACHE = {}


def _get_nc():
    if "nc" not in _NC_CACHE:
        nc = _build()
        orig_to_json = nc.to_json_bytes
        nc.to_json_bytes = lambda: _split_multi_waits(orig_to_json())
        _NC_CACHE["nc"] = nc
    return _NC_CACHE["nc"]


def kernel(x, _trace=False, _tmpdir=None):
    assert x.shape == (32, 512, 64, 64) and x.dtype == np.float32
    x_flat = np.ascontiguousarray(x.reshape(BC_TOTAL, L)).astype(NP_BF16)

    B64 = _blaschke_basis_f64(L)
    bn = np.ascontiguousarray(B64).astype(NP_BF16)          # [N, L]
    # packed bt: [128, KC*64] with bt[p, 64k+n] = B[n, 128k+p]
    bt = np.ascontiguousarray(
        B64.T.reshape(KC, 128, N_BASIS).transpose(1, 0, 2).reshape(128, KC * N_BASIS)
    ).astype(NP_BF16)
    ident = np.eye(128, dtype=np.float32)

    in_maps = []
    for i in range(N_CORES):
        shard = x_flat[i * BC:(i + 1) * BC]                 # [BC, L] bf16
        xt = np.ascontiguousarray(shard.T)                  # [L, BC] bf16
        in_maps.append({"xt": xt, "bt": bt, "bn": bn, "ident": ident})

    nc = _get_nc()
    res = run_bass_kernel_spmd(
        nc, in_maps, core_ids=list(range(N_CORES)), trace=_trace, tmpdir=_tmpdir
    )

    outs = []
    for i in range(N_CORES):
        outT = np.asarray(res.results[i]["out"])            # [L, BC] bf16
        outs.append(np.ascontiguousarray(outT.T).astype(np.float32))
    out = np.concatenate(outs, axis=0).reshape(32, 512, 64, 64)
    if _trace:
        return out, res
    return out
